# revision 1
# baseline (speedup 1.0000x reference)
"""Trainium2 kernel for nn_PolynomialLayer: out = [x, x_i*x_j (i<=j)] @ W.T + bias.

Data-parallel over batch across 8 NeuronCores. Each core:
  - receives x^T for its 1024-row batch shard ([128 feat, 1024 b]) plus 64
    partition-rotated copies (host np.roll; pure data movement),
  - builds the 8256 pairwise-product features on the vector engine as 65
    full-128-partition tensor_tensor multiplies (chunk d: xT * rot_d covers
    all index pairs with cyclic difference {d, 128-d}),
  - accumulates out^T[512, 1024] = sum_c Wc.T @ PTc on the tensor engine
    (66 K-chunks of 128, all 8 PSUM banks: 4 n-chunks x 2 b-chunks),
  - adds bias during the PSUM->SBUF copies (split scalar/vector engines).
The host pre-permutes/transposes the weight matrix so its column order
matches the on-chip feature-chunk layout.
"""

import os
import sys
import numpy as np

for _p in ("/opt/trn_rl_repo",):
    if os.path.isdir(_p) and _p not in sys.path:
        sys.path.append(_p)

B, D, NOUT = 8192, 128, 512
NCORES = 8
BC = B // NCORES            # 1024 batch rows per core
NCHUNK = 66                 # 1 linear + 1 squares + 64 rotation chunks
NROT = 64                   # rotation distances d=1..64
NB = BC // 512              # moving-operand chunks per core (2)
NN = NOUT // 128            # output partition chunks (4)

COMPUTE_DT = os.environ.get("POLY_COMPUTE_DT", "bfloat16")  # bfloat16 | mixed | float32r


def _ensure_axon_hooks_stub():
    """concourse's trace path imports antenv.axon_hooks; provide a stub if
    this image lacks it so an env-triggered trace degrades instead of
    crashing."""
    try:
        import antenv.axon_hooks  # noqa: F401
        return
    except Exception:
        pass
    try:
        import types
        import antenv
        m = types.ModuleType("antenv.axon_hooks")
        m._hook = None
        m.set_axon_ntff_profile_hook = lambda h: setattr(m, "_hook", h)
        m.get_axon_ntff_profile_hook = lambda: m._hook
        sys.modules["antenv.axon_hooks"] = m
        antenv.axon_hooks = m
    except Exception:
        pass


def _pair_index_map():
    """Map (chunk c, partition p) -> column index in the reference feature
    order (or -1 for padding).

    Reference order: [x_0..x_127] then pairs (i,j) i<=j in
    combinations_with_replacement order.
    Chunk layout: c=0 linear; c=1 squares; c=2..65 -> d=c-1 in 1..64 with
    (i,j) = sorted(p, (p+d) % 128); for d=64 only p<64 is valid.
    """
    idx = np.full((NCHUNK, D), -1, dtype=np.int64)
    off = 128 * np.arange(D) - (np.arange(D) * (np.arange(D) - 1)) // 2

    def pair_idx(i, j):
        return D + off[i] + (j - i)

    idx[0, :] = np.arange(D)
    p = np.arange(D)
    idx[1, :] = pair_idx(p, p)
    for d in range(1, NROT + 1):
        c = 1 + d
        q = (p + d) % D
        i = np.minimum(p, q)
        j = np.maximum(p, q)
        v = pair_idx(i, j)
        if d == NROT:
            v = np.where(p < 64, v, -1)
        idx[c, :] = v
    return idx


_nc_cache = None


def _build_nc():
    global _nc_cache
    if _nc_cache is not None:
        return _nc_cache
    import concourse.tile as tile
    from concourse import bacc, mybir

    # "mixed": f32r x/weights/products (precision), bf16 rotation streams (DMA)
    cdt = mybir.dt.float32r if COMPUTE_DT == "mixed" else getattr(mybir.dt, COMPUTE_DT)
    rdt = mybir.dt.bfloat16 if COMPUTE_DT == "mixed" else cdt
    nc = bacc.Bacc("TRN2", target_bir_lowering=False, debug=False)
    # partition-major DRAM layouts: one dma_start covers a GROUP of chunks
    # with large per-partition-contiguous descriptors.
    xT_ext = nc.dram_tensor("xT", [D, BC], cdt, kind="ExternalInput")
    rots_ext = nc.dram_tensor("rots", [D, NROT, BC], rdt, kind="ExternalInput")
    wp_ext = nc.dram_tensor("wp", [D, NCHUNK, NOUT], cdt, kind="ExternalInput")
    bias_ext = nc.dram_tensor("biasp", [D, NN], mybir.dt.float32, kind="ExternalInput")
    out_ext = nc.dram_tensor("out", [NOUT, BC], mybir.dt.float32, kind="ExternalOutput")

    # chunks per DMA group, small leading groups so the pipeline starts fast
    wg_sizes = [1, 1, 2, 4] + [6] * 9 + [4]          # sums to 66
    rg_sizes = [1, 1, 2, 4] + [6] * 9 + [2]          # sums to 64
    wg_starts = np.cumsum([0] + wg_sizes).tolist()
    rg_starts = np.cumsum([0] + rg_sizes).tolist()
    wg_of_chunk = {}
    for g, s in enumerate(wg_starts[:-1]):
        for c in range(s, wg_starts[g + 1]):
            wg_of_chunk[c] = g
    rg_of_rc = {}
    for g, s in enumerate(rg_starts[:-1]):
        for r in range(s, rg_starts[g + 1]):
            rg_of_rc[r] = g

    with tile.TileContext(nc) as tc:
        wide = COMPUTE_DT == "float32r"   # 4-byte rotations: tighter SBUF budget
        with (
            tc.tile_pool(name="xpool", bufs=1) as xpool,
            tc.tile_pool(name="wpool", bufs=4 if wide else 5) as wpool,
            tc.tile_pool(name="rpool", bufs=3 if wide else (4 if COMPUTE_DT == "mixed" else 6)) as rpool,
            tc.tile_pool(name="ppool", bufs=6 if COMPUTE_DT != "bfloat16" else 8) as ppool,
            tc.tile_pool(name="opool", bufs=1) as opool,
            tc.tile_pool(name="psum", bufs=1, space="PSUM") as psum,
        ):
            xT = xpool.tile([D, BC], cdt)
            nc.sync.dma_start(xT[:], xT_ext[:])

            ps = [[psum.tile([D, 512], mybir.dt.float32,
                             name=f"ps_{n}_{b}", tag=f"ps_{n}_{b}")
                   for b in range(NB)] for n in range(NN)]

            wg_tiles = {}
            rg_tiles = {}
            for c in range(NCHUNK):
                g = wg_of_chunk[c]
                if c == wg_starts[g]:
                    sz = wg_sizes[g]
                    wg = wpool.tile([D, sz * NOUT], cdt, name="wg", tag="wg")
                    nc.sync.dma_start(wg[:], wp_ext[:, c:c + sz, :])
                    wg_tiles[g] = wg
                rc = c - 2  # rotation index for this chunk
                if c >= 2:
                    rgi = rg_of_rc[rc]
                    if rc == rg_starts[rgi]:
                        sz = rg_sizes[rgi]
                        rg = rpool.tile([D, sz * BC], rdt, name="rg", tag="rg")
                        nc.sync.dma_start(rg[:], rots_ext[:, rc:rc + sz, :])
                        rg_tiles[rgi] = rg

                if c == 0:
                    mv = xT
                elif c == 1:
                    mv = ppool.tile([D, BC], cdt, name="pt", tag="pt")
                    nc.vector.tensor_mul(mv[:], xT[:], xT[:])
                else:
                    rg = rg_tiles[rg_of_rc[rc]]
                    roff = rc - rg_starts[rg_of_rc[rc]]
                    rslice = rg[:, roff * BC:(roff + 1) * BC]
                    mv = ppool.tile([D, BC], cdt, name="pt", tag="pt")
                    nc.vector.tensor_mul(mv[:], xT[:], rslice)
                wg = wg_tiles[g]
                woff = (c - wg_starts[g]) * NOUT
                for n in range(NN):
                    for b in range(NB):
                        nc.tensor.matmul(
                            ps[n][b][:],
                            wg[:, woff + n * 128:woff + (n + 1) * 128],
                            mv[:, b * 512:(b + 1) * 512],
                            start=(c == 0),
                            stop=(c == NCHUNK - 1),
                        )

            bias = xpool.tile([D, NN], mybir.dt.float32)
            nc.sync.dma_start(bias[:], bias_ext[:])
            obig = opool.tile([D, NN * NB * 512], mybir.dt.float32)
            for n in range(NN):
                for b in range(NB):
                    ot = obig[:, (n * NB + b) * 512:(n * NB + b + 1) * 512]
                    if b == 0:
                        nc.scalar.activation(
                            ot, ps[n][b][:],
                            mybir.ActivationFunctionType.Identity,
                            bias=bias[:, n:n + 1],
                        )
                    else:
                        nc.vector.tensor_scalar_add(ot, ps[n][b][:], bias[:, n:n + 1])
            # two halves so the first scatter overlaps the remaining copies
            h = NN // 2
            nc.sync.dma_start(
                out_ext[0:h * 128, :].rearrange("(n p) (b f) -> p n b f", n=h, b=NB),
                obig[:, 0:h * NB * 512].rearrange("p (n b f) -> p n b f", n=h, b=NB),
            )
            nc.sync.dma_start(
                out_ext[h * 128:, :].rearrange("(n p) (b f) -> p n b f", n=h, b=NB),
                obig[:, h * NB * 512:].rearrange("p (n b f) -> p n b f", n=h, b=NB),
            )

    nc.compile()
    _nc_cache = nc
    return nc


def _prep_inputs(x, weights, bias):
    if COMPUTE_DT == "bfloat16":
        import ml_dtypes
        cdt_np = np.dtype(ml_dtypes.bfloat16)
        rdt_np = cdt_np
    elif COMPUTE_DT == "mixed":
        import ml_dtypes
        cdt_np = np.dtype(np.float32)
        rdt_np = np.dtype(ml_dtypes.bfloat16)
    else:
        cdt_np = np.dtype(np.float32)
        rdt_np = cdt_np

    x = np.asarray(x, dtype=np.float32)
    weights = np.asarray(weights, dtype=np.float32)
    bias = np.asarray(bias, dtype=np.float32)

    idx = _pair_index_map()
    wcols = weights.T  # [8384, 512]
    wp = np.zeros((NCHUNK, D, NOUT), dtype=np.float32)
    valid = idx >= 0
    wp[valid] = wcols[idx[valid]]
    wp = np.ascontiguousarray(wp.transpose(1, 0, 2)).astype(cdt_np)  # [D, NCHUNK, NOUT]

    biasp = np.ascontiguousarray(bias.reshape(NN, 128).T)  # [128, NN] f32

    in_maps = []
    for k in range(NCORES):
        xs = np.ascontiguousarray(x[k * BC:(k + 1) * BC].T).astype(cdt_np)  # [128, BC]
        xr = xs.astype(rdt_np)
        rots = np.stack([np.roll(xr, -d, axis=0) for d in range(1, NROT + 1)])
        rots = rots.transpose(1, 0, 2)  # [D, NROT, BC] partition-major
        in_maps.append({
            "xT": xs,
            "rots": np.ascontiguousarray(rots),
            "wp": wp,
            "biasp": biasp,
        })
    return in_maps


def kernel(x, weights, bias):
    _ensure_axon_hooks_stub()
    from concourse.bass_utils import run_bass_kernel_spmd

    nc = _build_nc()
    in_maps = _prep_inputs(x, weights, bias)
    res = run_bass_kernel_spmd(nc, in_maps, core_ids=list(range(NCORES)))
    outT = np.concatenate([res.results[k]["out"] for k in range(NCORES)], axis=1)
    out = np.ascontiguousarray(outT.T, dtype=np.float32)  # [8192, 512]
    kernel.last_results = res
    return out



# revision 6
# speedup vs baseline: 1.1604x; 1.1604x over previous
"""Trainium2 kernel for nn_PolynomialLayer: out = [x, x_i*x_j (i<=j)] @ W.T + bias.

Data-parallel over batch across 8 NeuronCores. Each core:
  - receives x^T for its 1024-row batch shard ([128 feat, 1024 b]) plus 64
    partition-rotated copies (host np.roll; pure data movement),
  - builds the 8256 pairwise-product features on the vector engine as
    full-128-partition tensor_tensor multiplies (chunk d: xT * rot_d covers
    all index pairs with cyclic difference {d, 128-d}),
  - accumulates out^T[512, 1024] = sum_c Wc.T @ PTc on the tensor engine.
    The last M_PAIRS*2 cross chunks run as fp8e4 DoubleRow pairs (2 K-chunks
    per matmul pass, ~1.77x streaming rate); the rest stay bf16. The fp8
    fraction is sized so the fp8 quantization noise keeps the output rel-err
    under the accuracy budget.
  - bias is folded into the GEMM via an all-ones feature row in the padded
    d=64 chunk (its weights row carries the bias), so PSUM holds the final
    result, and
  - drains PSUM -> SBUF (bf16) -> DRAM per 128-row output group, pipelined.
Dummy warmup matmuls at kernel start lift the PE clock gate (HAM) to full
rate before the real stream begins.
The host pre-permutes/transposes the weight matrix so its column order
matches the on-chip feature-chunk layout.
"""

import os
import sys
import numpy as np

for _p in ("/opt/trn_rl_repo",):
    if os.path.isdir(_p) and _p not in sys.path:
        sys.path.append(_p)

B, D, NOUT = 8192, 128, 512
NCORES = 8
BC = B // NCORES            # 1024 batch rows per core
NCHUNK = 66                 # 1 linear + 1 squares + 64 rotation chunks
NROT = 64                   # rotation distances d=1..64
NB = BC // 512              # moving-operand chunks per core (2)
NN = NOUT // 128            # output partition chunks (4)

# fp8 DoubleRow pairs (2 cross chunks each), taken from the tail of the
# cross-chunk sequence. Error budget: rel_fro ~= 0.032 * sqrt(2*m*128/8640).
M_PAIRS = int(os.environ.get("POLY_M_PAIRS", "9"))
N_WARMUP = int(os.environ.get("POLY_WARMUP", "8"))

# chunk processing order: [linear, squares, d64+bias] + bf16 cross + fp8 pairs
# cross chunks are c=2..65 <-> d=c-1; c65 (d=64, half-padded) carries the
# all-ones bias row so it must stay bf16 and precedes the fp8 tail.
_FP8_CHUNKS = list(range(65 - 2 * M_PAIRS, 65))          # 2m chunks, d<=63
_BF16_CROSS = list(range(2, 65 - 2 * M_PAIRS))           # d=1..63-2m
PROC = [0, 1, 65] + _BF16_CROSS + _FP8_CHUNKS
CROSS_ORDER = [c for c in PROC if c >= 2]                # rot slot order
NBF = 3 + len(_BF16_CROSS)                               # bf16 weight chunks
BIAS_ROW = 64                                            # ones row in c65


def _ensure_axon_hooks_stub():
    """concourse's trace path imports antenv.axon_hooks; provide a stub if
    this image lacks it so an env-triggered trace degrades instead of
    crashing."""
    try:
        import antenv.axon_hooks  # noqa: F401
        return
    except Exception:
        pass
    try:
        import types
        import antenv
        m = types.ModuleType("antenv.axon_hooks")
        m._hook = None
        m.set_axon_ntff_profile_hook = lambda h: setattr(m, "_hook", h)
        m.get_axon_ntff_profile_hook = lambda: m._hook
        sys.modules["antenv.axon_hooks"] = m
        antenv.axon_hooks = m
    except Exception:
        pass


def _pair_index_map():
    """Map (chunk c, partition p) -> column index in the reference feature
    order (or -1 for padding).

    Reference order: [x_0..x_127] then pairs (i,j) i<=j in
    combinations_with_replacement order.
    Chunk layout: c=0 linear; c=1 squares; c=2..65 -> d=c-1 in 1..64 with
    (i,j) = sorted(p, (p+d) % 128); for d=64 only p<64 is valid.
    """
    idx = np.full((NCHUNK, D), -1, dtype=np.int64)
    off = 128 * np.arange(D) - (np.arange(D) * (np.arange(D) - 1)) // 2

    def pair_idx(i, j):
        return D + off[i] + (j - i)

    idx[0, :] = np.arange(D)
    p = np.arange(D)
    idx[1, :] = pair_idx(p, p)
    for d in range(1, NROT + 1):
        c = 1 + d
        q = (p + d) % D
        i = np.minimum(p, q)
        j = np.maximum(p, q)
        v = pair_idx(i, j)
        if d == NROT:
            v = np.where(p < 64, v, -1)
        idx[c, :] = v
    return idx


def _group_sizes(total, leading=(1, 1, 2, 4), body=6):
    """DMA group sizes: small leading groups so the pipeline starts fast."""
    sizes = []
    for s in leading:
        if sum(sizes) + s > total:
            break
        sizes.append(s)
    while sum(sizes) < total:
        sizes.append(min(body, total - sum(sizes)))
    return sizes


_nc_cache = None


def _build_nc():
    global _nc_cache
    if _nc_cache is not None:
        return _nc_cache
    import concourse.tile as tile
    from concourse import bacc, mybir

    bdt = mybir.dt.bfloat16
    fdt = mybir.dt.float8e4
    nc = bacc.Bacc("TRN2", target_bir_lowering=False, debug=False)
    # partition-major DRAM layouts: one dma_start covers a GROUP of chunks
    # with large per-partition-contiguous descriptors.
    xT_ext = nc.dram_tensor("xT", [D, BC], bdt, kind="ExternalInput")
    rots_ext = nc.dram_tensor("rots", [D, NROT, BC], bdt, kind="ExternalInput")
    wb_ext = nc.dram_tensor("wb", [D, NBF, NOUT], bdt, kind="ExternalInput")
    w8_ext = None
    if M_PAIRS:
        w8_ext = nc.dram_tensor("w8", [D, M_PAIRS, 2, NOUT], fdt,
                                kind="ExternalInput")
    out_ext = nc.dram_tensor("out", [NOUT, BC], bdt, kind="ExternalOutput")

    wg_sizes = _group_sizes(NBF)
    rg_sizes = _group_sizes(NROT)
    w8g_sizes = _group_sizes(M_PAIRS, leading=(2,), body=4) if M_PAIRS else []
    wg_starts = np.cumsum([0] + wg_sizes).tolist()
    rg_starts = np.cumsum([0] + rg_sizes).tolist()
    w8g_starts = np.cumsum([0] + w8g_sizes).tolist()

    def group_of(starts, i):
        for g in range(len(starts) - 1):
            if starts[g] <= i < starts[g + 1]:
                return g
        raise AssertionError

    # per-PROC-position metadata
    cross_pos = {c: k for k, c in enumerate(CROSS_ORDER)}   # rot slot
    bf16_pos = {}                                           # wb slot
    k = 0
    for c in PROC:
        if c not in _FP8_CHUNKS:
            bf16_pos[c] = k
            k += 1

    with tile.TileContext(nc) as tc:
        with (
            tc.tile_pool(name="xpool", bufs=1) as xpool,
            tc.tile_pool(name="wpool", bufs=4) as wpool,
            tc.tile_pool(name="w8pool", bufs=2) as w8pool,
            tc.tile_pool(name="rpool", bufs=4) as rpool,
            tc.tile_pool(name="ppool", bufs=6) as ppool,
            tc.tile_pool(name="p8pool", bufs=4) as p8pool,
            tc.tile_pool(name="opool", bufs=1) as opool,
            tc.tile_pool(name="psum", bufs=1, space="PSUM") as psum,
        ):
            ps = [[psum.tile([D, 512], mybir.dt.float32,
                             name=f"ps_{n}_{b}", tag=f"ps_{n}_{b}")
                   for b in range(NB)] for n in range(NN)]

            # ---- PE warmup: garbage matmuls to lift the HAM clock gate ----
            if N_WARMUP:
                junk = xpool.tile([D, 512], bdt)
                nc.vector.memset(junk[:], 0.0)
                for _ in range(N_WARMUP):
                    nc.tensor.matmul(ps[0][0][:], junk[:, 0:128], junk[:],
                                     start=True, stop=True,
                                     skip_group_check=True)

            xT = xpool.tile([D, BC], bdt)
            # two halves so chunk 0's b=0 matmuls start before the second
            # half lands
            nc.sync.dma_start(xT[:, 0:512], xT_ext[:, 0:512])
            nc.sync.dma_start(xT[:, 512:], xT_ext[:, 512:])

            wg_tiles = {}
            rg_tiles = {}
            w8g_tiles = {}

            def fetch_wb(slot):
                g = group_of(wg_starts, slot)
                if slot == wg_starts[g]:
                    sz = wg_sizes[g]
                    wg = wpool.tile([D, sz, NOUT], bdt, name="wg", tag="wg")
                    nc.sync.dma_start(wg[:], wb_ext[:, slot:slot + sz, :])
                    wg_tiles[g] = wg
                return wg_tiles[g], slot - wg_starts[g]

            def fetch_rot(slot):
                g = group_of(rg_starts, slot)
                if slot == rg_starts[g]:
                    sz = rg_sizes[g]
                    rg = rpool.tile([D, sz, BC], bdt, name="rg", tag="rg")
                    nc.sync.dma_start(rg[:], rots_ext[:, slot:slot + sz, :])
                    rg_tiles[g] = rg
                rg = rg_tiles[g]
                return rg[:, slot - rg_starts[g], :]

            def fetch_w8(pair):
                g = group_of(w8g_starts, pair)
                if pair == w8g_starts[g]:
                    sz = w8g_sizes[g]
                    wg = w8pool.tile([D, sz, 2, NOUT], fdt, name="w8g",
                                     tag="w8g")
                    nc.sync.dma_start(wg[:], w8_ext[:, pair:pair + sz, :, :])
                    w8g_tiles[g] = wg
                return w8g_tiles[g], pair - w8g_starts[g]

            # prefetch order interleaves weight/rot groups as consumed below.
            first = PROC[0]
            last = PROC[-1]
            i = 0
            while i < len(PROC):
                c = PROC[i]
                if c in _FP8_CHUNKS:
                    pair = (i - (len(PROC) - 2 * M_PAIRS)) // 2
                    cA, cB = PROC[i], PROC[i + 1]
                    w8g, woff = fetch_w8(pair)
                    rsA = fetch_rot(cross_pos[cA])
                    rsB = fetch_rot(cross_pos[cB])
                    pp = p8pool.tile([D, 2, BC], fdt, name="p8", tag="p8")
                    nc.vector.tensor_mul(pp[:, 0, :], xT[:], rsA)
                    nc.vector.tensor_mul(pp[:, 1, :], xT[:], rsB)
                    for n in range(NN):
                        for b in range(NB):
                            nc.tensor.matmul(
                                ps[n][b][:],
                                w8g[:, woff, :, n * 128:(n + 1) * 128],
                                pp[:, :, b * 512:(b + 1) * 512],
                                start=(cA == first),
                                stop=(cB == last),
                                perf_mode=mybir.MatmulPerfMode.DoubleRow,
                            )
                    i += 2
                    continue

                wg, woff = fetch_wb(bf16_pos[c])
                if c == 0:
                    mv = xT
                elif c == 1:
                    mv = ppool.tile([D, BC], bdt, name="pt", tag="pt")
                    nc.vector.tensor_mul(mv[:], xT[:], xT[:])
                else:
                    rslice = fetch_rot(cross_pos[c])
                    mv = ppool.tile([D, BC], bdt, name="pt", tag="pt")
                    nc.vector.tensor_mul(mv[:], xT[:], rslice)
                    if c == 65:
                        # all-ones feature row: its weight row carries bias
                        nc.gpsimd.memset(mv[BIAS_ROW:BIAS_ROW + 1, :], 1.0)
                for n in range(NN):
                    for b in range(NB):
                        nc.tensor.matmul(
                            ps[n][b][:],
                            wg[:, woff, n * 128:(n + 1) * 128],
                            mv[:, b * 512:(b + 1) * 512],
                            start=(c == first),
                            stop=(c == last),
                        )
                i += 1

            # drain: PSUM -> SBUF bf16 (scalar engine b=0, vector b=1 so the
            # two banks of a group copy in parallel), then one DMA per group.
            for n in range(NN):
                ob = opool.tile([D, NB * 512], bdt, name=f"ob{n}",
                                tag=f"ob{n}")
                nc.scalar.activation(ob[:, 0:512], ps[n][0][:],
                                     mybir.ActivationFunctionType.Identity)
                nc.vector.tensor_copy(ob[:, 512:1024], ps[n][1][:])
                nc.sync.dma_start(
                    out_ext[n * 128:(n + 1) * 128, :]
                    .rearrange("p (b f) -> p b f", b=NB),
                    ob[:].rearrange("p (b f) -> p b f", b=NB),
                )

    nc.compile()
    _nc_cache = nc
    return nc


def _prep_inputs(x, weights, bias):
    import ml_dtypes
    bdt_np = np.dtype(ml_dtypes.bfloat16)
    fdt_np = np.dtype(ml_dtypes.float8_e4m3)

    x = np.asarray(x, dtype=np.float32)
    weights = np.asarray(weights, dtype=np.float32)
    bias = np.asarray(bias, dtype=np.float32)

    idx = _pair_index_map()
    wcols = weights.T  # [8384, 512]
    wp = np.zeros((NCHUNK, D, NOUT), dtype=np.float32)
    valid = idx >= 0
    wp[valid] = wcols[idx[valid]]
    wp[65, BIAS_ROW, :] = bias  # ones-row bias fold (c65 row 64 is padding)

    wb = np.stack([wp[c] for c in PROC if c not in _FP8_CHUNKS])
    wb = np.ascontiguousarray(wb.transpose(1, 0, 2)).astype(bdt_np)
    w8 = None
    if M_PAIRS:
        w8 = np.stack([wp[c] for c in _FP8_CHUNKS])  # [2m, D, NOUT]
        w8 = w8.reshape(M_PAIRS, 2, D, NOUT).transpose(2, 0, 1, 3)
        w8 = np.ascontiguousarray(w8).astype(fdt_np)  # [D, m, 2, NOUT]

    in_maps = []
    for k in range(NCORES):
        xs = np.ascontiguousarray(x[k * BC:(k + 1) * BC].T).astype(bdt_np)
        rots = np.stack([np.roll(xs, -(c - 1), axis=0) for c in CROSS_ORDER])
        rots = rots.transpose(1, 0, 2)  # [D, NROT, BC] partition-major
        im = {
            "xT": xs,
            "rots": np.ascontiguousarray(rots),
            "wb": wb,
        }
        if M_PAIRS:
            im["w8"] = w8
        in_maps.append(im)
    return in_maps


def kernel(x, weights, bias):
    _ensure_axon_hooks_stub()
    from concourse.bass_utils import run_bass_kernel_spmd

    nc = _build_nc()
    in_maps = _prep_inputs(x, weights, bias)
    res = run_bass_kernel_spmd(nc, in_maps, core_ids=list(range(NCORES)))
    outT = np.concatenate(
        [np.asarray(res.results[k]["out"]) for k in range(NCORES)], axis=1)
    out = np.ascontiguousarray(outT.T.astype(np.float32))  # [8192, 512]
    kernel.last_results = res
    return out


# revision 9
# speedup vs baseline: 1.1613x; 1.0007x over previous
"""Trainium2 kernel for nn_PolynomialLayer: out = [x, x_i*x_j (i<=j)] @ W.T + bias.

Data-parallel over batch across 8 NeuronCores. Each core:
  - receives x^T for its 1024-row batch shard ([128 feat, 1024 b]) plus 64
    partition-rotated copies (host np.roll; pure data movement),
  - builds the 8256 pairwise-product features on the vector engine as
    full-128-partition tensor_tensor multiplies (chunk d: xT * rot_d covers
    all index pairs with cyclic difference {d, 128-d}),
  - accumulates out^T[512, 1024] = sum_c Wc.T @ PTc on the tensor engine.
    The last M_PAIRS*2 cross chunks run as fp8e4 DoubleRow pairs (2 K-chunks
    per matmul pass, ~1.77x streaming rate); the rest stay bf16. The fp8
    fraction is sized so the fp8 quantization noise keeps the output rel-err
    under the accuracy budget.
  - bias is folded into the GEMM via an all-ones feature row in the padded
    d=64 chunk (its weights row carries the bias), so PSUM holds the final
    result, and
  - drains PSUM -> SBUF (bf16) -> DRAM per 128-row output group, pipelined.
Dummy warmup matmuls at kernel start lift the PE clock gate (HAM) to full
rate before the real stream begins.
The host pre-permutes/transposes the weight matrix so its column order
matches the on-chip feature-chunk layout.
"""

import os
import sys
import numpy as np

for _p in ("/opt/trn_rl_repo",):
    if os.path.isdir(_p) and _p not in sys.path:
        sys.path.append(_p)

B, D, NOUT = 8192, 128, 512
NCORES = 8
BC = B // NCORES            # 1024 batch rows per core
NCHUNK = 66                 # 1 linear + 1 squares + 64 rotation chunks
NROT = 64                   # rotation distances d=1..64
NB = BC // 512              # moving-operand chunks per core (2)
NN = NOUT // 128            # output partition chunks (4)

# fp8 DoubleRow pairs (2 cross chunks each), taken from the tail of the
# cross-chunk sequence. Error budget: rel_fro ~= 0.032 * sqrt(2*m*128/8640).
M_PAIRS = int(os.environ.get("POLY_M_PAIRS", "9"))
N_WARMUP = int(os.environ.get("POLY_WARMUP", "4"))
ROT_BODY = int(os.environ.get("POLY_ROT_BODY", "4"))

# chunk processing order: [linear, squares, d64+bias] + bf16 cross + fp8 pairs
# cross chunks are c=2..65 <-> d=c-1; c65 (d=64, half-padded) carries the
# all-ones bias row so it must stay bf16 and precedes the fp8 tail.
_FP8_CHUNKS = list(range(65 - 2 * M_PAIRS, 65))          # 2m chunks, d<=63
_BF16_CROSS = list(range(2, 65 - 2 * M_PAIRS))           # d=1..63-2m
PROC = [0, 1, 65] + _BF16_CROSS + _FP8_CHUNKS
CROSS_ORDER = [c for c in PROC if c >= 2]                # rot slot order
NBF = 3 + len(_BF16_CROSS)                               # bf16 weight chunks
BIAS_ROW = 64                                            # ones row in c65


def _ensure_axon_hooks_stub():
    """concourse's trace path imports antenv.axon_hooks; provide a stub if
    this image lacks it so an env-triggered trace degrades instead of
    crashing."""
    try:
        import antenv.axon_hooks  # noqa: F401
        return
    except Exception:
        pass
    try:
        import types
        import antenv
        m = types.ModuleType("antenv.axon_hooks")
        m._hook = None
        m.set_axon_ntff_profile_hook = lambda h: setattr(m, "_hook", h)
        m.get_axon_ntff_profile_hook = lambda: m._hook
        sys.modules["antenv.axon_hooks"] = m
        antenv.axon_hooks = m
    except Exception:
        pass


def _pair_index_map():
    """Map (chunk c, partition p) -> column index in the reference feature
    order (or -1 for padding).

    Reference order: [x_0..x_127] then pairs (i,j) i<=j in
    combinations_with_replacement order.
    Chunk layout: c=0 linear; c=1 squares; c=2..65 -> d=c-1 in 1..64 with
    (i,j) = sorted(p, (p+d) % 128); for d=64 only p<64 is valid.
    """
    idx = np.full((NCHUNK, D), -1, dtype=np.int64)
    off = 128 * np.arange(D) - (np.arange(D) * (np.arange(D) - 1)) // 2

    def pair_idx(i, j):
        return D + off[i] + (j - i)

    idx[0, :] = np.arange(D)
    p = np.arange(D)
    idx[1, :] = pair_idx(p, p)
    for d in range(1, NROT + 1):
        c = 1 + d
        q = (p + d) % D
        i = np.minimum(p, q)
        j = np.maximum(p, q)
        v = pair_idx(i, j)
        if d == NROT:
            v = np.where(p < 64, v, -1)
        idx[c, :] = v
    return idx


def _group_sizes(total, leading=(1, 1, 2, 4), body=6):
    """DMA group sizes: small leading groups so the pipeline starts fast."""
    sizes = []
    for s in leading:
        if sum(sizes) + s > total:
            break
        sizes.append(s)
    while sum(sizes) < total:
        sizes.append(min(body, total - sum(sizes)))
    return sizes


_nc_cache = None


def _build_nc():
    global _nc_cache
    if _nc_cache is not None:
        return _nc_cache
    import concourse.tile as tile
    from concourse import bacc, mybir

    bdt = mybir.dt.bfloat16
    fdt = mybir.dt.float8e4
    nc = bacc.Bacc("TRN2", target_bir_lowering=False, debug=False)
    # partition-major DRAM layouts: one dma_start covers a GROUP of chunks
    # with large per-partition-contiguous descriptors.
    xT_ext = nc.dram_tensor("xT", [D, BC], bdt, kind="ExternalInput")
    rots_ext = nc.dram_tensor("rots", [D, NROT, BC], bdt, kind="ExternalInput")
    wb_ext = nc.dram_tensor("wb", [D, NBF, NOUT], bdt, kind="ExternalInput")
    w8_ext = None
    if M_PAIRS:
        w8_ext = nc.dram_tensor("w8", [D, M_PAIRS, 2, NOUT], fdt,
                                kind="ExternalInput")
    out_ext = nc.dram_tensor("out", [NOUT, BC], bdt, kind="ExternalOutput")

    wg_sizes = _group_sizes(NBF)
    rg_sizes = _group_sizes(NROT, body=ROT_BODY)
    w8g_sizes = _group_sizes(M_PAIRS, leading=(2,), body=4) if M_PAIRS else []
    wg_starts = np.cumsum([0] + wg_sizes).tolist()
    rg_starts = np.cumsum([0] + rg_sizes).tolist()
    w8g_starts = np.cumsum([0] + w8g_sizes).tolist()

    def group_of(starts, i):
        for g in range(len(starts) - 1):
            if starts[g] <= i < starts[g + 1]:
                return g
        raise AssertionError

    # per-PROC-position metadata
    cross_pos = {c: k for k, c in enumerate(CROSS_ORDER)}   # rot slot
    bf16_pos = {}                                           # wb slot
    k = 0
    for c in PROC:
        if c not in _FP8_CHUNKS:
            bf16_pos[c] = k
            k += 1

    with tile.TileContext(nc) as tc:
        with (
            tc.tile_pool(name="xpool", bufs=1) as xpool,
            tc.tile_pool(name="wpool", bufs=4) as wpool,
            tc.tile_pool(name="w8pool", bufs=2) as w8pool,
            tc.tile_pool(name="rpool", bufs=6) as rpool,
            tc.tile_pool(name="ppool", bufs=8) as ppool,
            tc.tile_pool(name="p8pool", bufs=4) as p8pool,
            tc.tile_pool(name="opool", bufs=1) as opool,
            tc.tile_pool(name="psum", bufs=1, space="PSUM") as psum,
        ):
            ps = [[psum.tile([D, 512], mybir.dt.float32,
                             name=f"ps_{n}_{b}", tag=f"ps_{n}_{b}")
                   for b in range(NB)] for n in range(NN)]

            # ---- PE warmup: garbage matmuls to lift the HAM clock gate ----
            if N_WARMUP:
                junk = xpool.tile([D, 512], bdt)
                nc.vector.memset(junk[:], 0.0)
                for _ in range(N_WARMUP):
                    nc.tensor.matmul(ps[0][0][:], junk[:, 0:128], junk[:],
                                     start=True, stop=True,
                                     skip_group_check=True)

            xT = xpool.tile([D, BC], bdt)
            # two halves so chunk 0's b=0 matmuls start before the second
            # half lands
            nc.sync.dma_start(xT[:, 0:512], xT_ext[:, 0:512])
            nc.sync.dma_start(xT[:, 512:], xT_ext[:, 512:])

            wg_tiles = {}
            rg_tiles = {}
            w8g_tiles = {}

            def fetch_wb(slot):
                g = group_of(wg_starts, slot)
                if slot == wg_starts[g]:
                    sz = wg_sizes[g]
                    wg = wpool.tile([D, sz, NOUT], bdt, name="wg", tag="wg")
                    nc.sync.dma_start(wg[:], wb_ext[:, slot:slot + sz, :])
                    wg_tiles[g] = wg
                return wg_tiles[g], slot - wg_starts[g]

            def fetch_rot(slot):
                g = group_of(rg_starts, slot)
                if slot == rg_starts[g]:
                    sz = rg_sizes[g]
                    rg = rpool.tile([D, sz, BC], bdt, name="rg", tag="rg")
                    nc.sync.dma_start(rg[:], rots_ext[:, slot:slot + sz, :])
                    rg_tiles[g] = rg
                rg = rg_tiles[g]
                return rg[:, slot - rg_starts[g], :]

            def fetch_w8(pair):
                g = group_of(w8g_starts, pair)
                if pair == w8g_starts[g]:
                    sz = w8g_sizes[g]
                    wg = w8pool.tile([D, sz, 2, NOUT], fdt, name="w8g",
                                     tag="w8g")
                    nc.sync.dma_start(wg[:], w8_ext[:, pair:pair + sz, :, :])
                    w8g_tiles[g] = wg
                return w8g_tiles[g], pair - w8g_starts[g]

            # prefetch order interleaves weight/rot groups as consumed below.
            first = PROC[0]
            last = PROC[-1]
            i = 0
            while i < len(PROC):
                c = PROC[i]
                if c in _FP8_CHUNKS:
                    pair = (i - (len(PROC) - 2 * M_PAIRS)) // 2
                    cA, cB = PROC[i], PROC[i + 1]
                    w8g, woff = fetch_w8(pair)
                    rsA = fetch_rot(cross_pos[cA])
                    rsB = fetch_rot(cross_pos[cB])
                    pp = p8pool.tile([D, 2, BC], fdt, name="p8", tag="p8")
                    nc.vector.tensor_mul(pp[:, 0, :], xT[:], rsA)
                    nc.vector.tensor_mul(pp[:, 1, :], xT[:], rsB)
                    for n in range(NN):
                        for b in range(NB):
                            nc.tensor.matmul(
                                ps[n][b][:],
                                w8g[:, woff, :, n * 128:(n + 1) * 128],
                                pp[:, :, b * 512:(b + 1) * 512],
                                start=(cA == first),
                                stop=(cB == last),
                                perf_mode=mybir.MatmulPerfMode.DoubleRow,
                            )
                    i += 2
                    continue

                wg, woff = fetch_wb(bf16_pos[c])
                if c == 0:
                    mv = xT
                elif c == 1:
                    mv = ppool.tile([D, BC], bdt, name="pt", tag="pt")
                    nc.vector.tensor_mul(mv[:], xT[:], xT[:])
                else:
                    rslice = fetch_rot(cross_pos[c])
                    mv = ppool.tile([D, BC], bdt, name="pt", tag="pt")
                    nc.vector.tensor_mul(mv[:], xT[:], rslice)
                    if c == 65:
                        # all-ones feature row: its weight row carries bias
                        nc.gpsimd.memset(mv[BIAS_ROW:BIAS_ROW + 1, :], 1.0)
                for n in range(NN):
                    for b in range(NB):
                        nc.tensor.matmul(
                            ps[n][b][:],
                            wg[:, woff, n * 128:(n + 1) * 128],
                            mv[:, b * 512:(b + 1) * 512],
                            start=(c == first),
                            stop=(c == last),
                        )
                i += 1

            # drain: PSUM -> SBUF bf16 (scalar engine b=0, vector b=1 so the
            # two banks of a group copy in parallel), then one DMA per group.
            for n in range(NN):
                ob = opool.tile([D, NB * 512], bdt, name=f"ob{n}",
                                tag=f"ob{n}")
                nc.scalar.activation(ob[:, 0:512], ps[n][0][:],
                                     mybir.ActivationFunctionType.Identity)
                nc.vector.tensor_copy(ob[:, 512:1024], ps[n][1][:])
                nc.sync.dma_start(
                    out_ext[n * 128:(n + 1) * 128, :]
                    .rearrange("p (b f) -> p b f", b=NB),
                    ob[:].rearrange("p (b f) -> p b f", b=NB),
                )

    nc.compile()
    _nc_cache = nc
    return nc


def _prep_inputs(x, weights, bias):
    import ml_dtypes
    bdt_np = np.dtype(ml_dtypes.bfloat16)
    fdt_np = np.dtype(ml_dtypes.float8_e4m3)

    x = np.asarray(x, dtype=np.float32)
    weights = np.asarray(weights, dtype=np.float32)
    bias = np.asarray(bias, dtype=np.float32)

    idx = _pair_index_map()
    wcols = weights.T  # [8384, 512]
    wp = np.zeros((NCHUNK, D, NOUT), dtype=np.float32)
    valid = idx >= 0
    wp[valid] = wcols[idx[valid]]
    wp[65, BIAS_ROW, :] = bias  # ones-row bias fold (c65 row 64 is padding)

    wb = np.stack([wp[c] for c in PROC if c not in _FP8_CHUNKS])
    wb = np.ascontiguousarray(wb.transpose(1, 0, 2)).astype(bdt_np)
    w8 = None
    if M_PAIRS:
        w8 = np.stack([wp[c] for c in _FP8_CHUNKS])  # [2m, D, NOUT]
        w8 = w8.reshape(M_PAIRS, 2, D, NOUT).transpose(2, 0, 1, 3)
        w8 = np.ascontiguousarray(w8).astype(fdt_np)  # [D, m, 2, NOUT]

    in_maps = []
    for k in range(NCORES):
        xs = np.ascontiguousarray(x[k * BC:(k + 1) * BC].T).astype(bdt_np)
        rots = np.stack([np.roll(xs, -(c - 1), axis=0) for c in CROSS_ORDER])
        rots = rots.transpose(1, 0, 2)  # [D, NROT, BC] partition-major
        im = {
            "xT": xs,
            "rots": np.ascontiguousarray(rots),
            "wb": wb,
        }
        if M_PAIRS:
            im["w8"] = w8
        in_maps.append(im)
    return in_maps


def kernel(x, weights, bias):
    _ensure_axon_hooks_stub()
    from concourse.bass_utils import run_bass_kernel_spmd

    nc = _build_nc()
    in_maps = _prep_inputs(x, weights, bias)
    res = run_bass_kernel_spmd(nc, in_maps, core_ids=list(range(NCORES)))
    outT = np.concatenate(
        [np.asarray(res.results[k]["out"]) for k in range(NCORES)], axis=1)
    out = np.ascontiguousarray(outT.T.astype(np.float32))  # [8192, 512]
    kernel.last_results = res
    return out
